# revision 7
# baseline (speedup 1.0000x reference)
"""Allegro-style GNN edge kernel on 8 TRN2 NeuronCores.

Strategy: the reference only ever uses edge_index[0] (edge_center) — the env
scatter-sum groups edges by center atom. We sort edges by center on the host
and pack whole atom-runs into 128-edge blocks (an atom's edges never straddle
a block). Blocks are dealt to the 8 cores, so the segment-sum + gather
becomes, per 128-edge block, multiplication by a block-diagonal 0/1 matrix
(B - I), built on the host — fully core-local, zero collectives.

Device layout is feature-major ([feat_dim partitions, edges free]); all MLP
matmuls use float32r (full-rate fp32, tf32-class precision) with weights
stationary. Cross-partition-block reductions/broadcasts in the tensor
products run as matmuls against small host-built constant matrices (engines
require partition-aligned operands). The (B-I) env matmul runs in bf16.
"""

import sys
import numpy as np

sys.path.insert(0, "/opt/trn_rl_repo")

N_ATOMS = 16000
F = 32
INV = 64
LAT = 256
NCORES = 8
T = 512          # edges per compute tile (matmul moving dim)
BLK = 128        # edges per env block
SQRT3 = 3.0 ** 0.5
RSQRT2 = 2.0 ** -0.5
ENV_C = float(1.0 / np.sqrt(16.0 - 1.0))


# ---------------------------------------------------------------- host prep

def _pack_edges(edge_center):
    """Sort edges by center atom; pack whole atom-runs into 128-edge blocks."""
    E = edge_center.shape[0]
    counts = np.bincount(edge_center, minlength=N_ATOMS)
    order_sorted = np.argsort(edge_center, kind="stable")

    atom_block = np.zeros(N_ATOMS, dtype=np.int64)
    atom_off = np.zeros(N_ATOMS, dtype=np.int64)
    blk = 0
    fill = 0
    for a in range(N_ATOMS):
        c = counts[a]
        if c == 0:
            continue
        if c > BLK:
            raise ValueError(f"atom {a} has {c} > {BLK} edges")
        if fill + c > BLK:
            blk += 1
            fill = 0
        atom_block[a] = blk
        atom_off[a] = fill
        fill += c
    n_blocks = blk + 1

    cum = np.concatenate([[0], np.cumsum(counts)])[:-1]
    run_start = np.repeat(cum, counts)
    within = np.arange(E) - run_start
    sc = edge_center[order_sorted]
    slot = atom_block[sc] * BLK + atom_off[sc] + within
    return slot, order_sorted, n_blocks


def _prep_host(inputs):
    """Shard + reorder inputs. Returns (in_maps, perms, Es)."""
    import ml_dtypes

    edge_center = np.asarray(inputs["edge_index"])[0].astype(np.int64)
    edge_attr = np.asarray(inputs["edge_attr"], dtype=np.float32)
    edge_inv = np.asarray(inputs["edge_inv"], dtype=np.float32)

    slot, order_sorted, n_blocks = _pack_edges(edge_center)

    nb_core = -(-n_blocks // NCORES)            # ceil
    nb_core = -(-nb_core // 4) * 4              # multiple of 4 (tile = 4 blocks)
    Es = nb_core * BLK
    nb_pad = nb_core * NCORES
    total = nb_pad * BLK

    perm = np.full(total, -1, dtype=np.int64)   # slot -> original edge id
    perm[slot] = order_sorted
    centers = np.full(total, -1, dtype=np.int64)
    centers[slot] = edge_center[order_sorted]

    xinv = np.zeros((total, INV), dtype=np.float32)
    attr = np.zeros((total, 4), dtype=np.float32)
    valid = perm >= 0
    xinv[valid] = edge_inv[perm[valid]]
    attr[valid] = edge_attr[perm[valid]]

    # attr replicated across the 32 features: row 32c+f = attr[:, c]
    arep = np.repeat(attr.T, F, axis=0)          # [128, total]

    # per-block (B - I) matrices, bf16
    cb = centers.reshape(nb_pad, BLK)
    M = (cb[:, :, None] == cb[:, None, :])
    M &= (cb >= 0)[:, :, None]
    M = M.astype(np.float32)
    idx = np.arange(BLK)
    M[:, idx, idx] = 0.0
    M = M.astype(ml_dtypes.bfloat16)

    # ---- weights: fold fan-in norms, ENV_C, 1/sqrt3, resnet RSQRT2 ----
    g = lambda k: np.asarray(inputs[k], dtype=np.float32)
    r = RSQRT2
    W1 = g("W_tb1") / np.sqrt(INV)
    W2 = g("W_tb2") / np.sqrt(LAT)
    We0 = g("W_env0") / np.sqrt(LAT)
    ew0, ew1 = We0[:, 0:64:2], We0[:, 1:64:2]
    nw0, nw1 = We0[:, 64::2], We0[:, 65::2]
    We0rep = np.concatenate([ew0, ew1, ew1, ew1, nw0, nw1, nw1, nw1], axis=1)
    Wl1a = g("W_l1a") / np.sqrt(LAT + F)
    Wl1b = g("W_l1b") / np.sqrt(LAT)
    We1 = g("W_env1") / np.sqrt(LAT) * r
    m0, m1 = We1[:, 0::2], We1[:, 1::2]
    We1rep = np.concatenate([m0, m1, m1, m1], axis=1)
    Wf1 = g("W_f1") / np.sqrt(LAT + F)
    Wf1 = Wf1.copy()
    Wf1[:LAT] *= r                               # lat input arrives unscaled (u1)
    Wf2 = g("W_f2") / np.sqrt(LAT) * r

    wtp0 = g("w_tp0")                            # [4, F]
    wtp1 = g("w_tp1")                            # [2, F]
    idf = np.arange(F)
    L0 = np.zeros((BLK, F), dtype=np.float32)
    L0[idf, idf] = wtp0[0] * ENV_C
    for c in range(1, 4):
        L0[c * F + idf, idf] = wtp0[1] * ENV_C / SQRT3
    L1 = np.zeros((BLK, F), dtype=np.float32)
    L1[idf, idf] = wtp1[0] * ENV_C
    for c in range(1, 4):
        L1[c * F + idf, idf] = wtp1[1] * ENV_C / SQRT3
    Prep = np.zeros((BLK, BLK), dtype=np.float32)
    for c in range(4):
        Prep[idf, c * F + idf] = 1.0
    tpv = np.zeros((BLK, 2), dtype=np.float32)   # col0=w3', col1=w2'
    for c in range(1, 4):
        tpv[c * F + idf, 0] = wtp0[3] * ENV_C
        tpv[c * F + idf, 1] = wtp0[2] * ENV_C
    ident = np.eye(BLK, dtype=np.float32).astype(ml_dtypes.bfloat16)

    weights = {
        "W1": W1, "W2": W2, "We0rep": We0rep, "Wl1a": Wl1a, "Wl1b": Wl1b,
        "We1rep": We1rep, "Wf1": Wf1, "Wf2": Wf2,
        "L0": L0, "L1": L1, "Prep": Prep, "tpv": tpv, "ident": ident,
    }
    weights = {k: np.ascontiguousarray(v) for k, v in weights.items()}

    in_maps, perms = [], []
    for c in range(NCORES):
        sl = slice(c * Es, (c + 1) * Es)
        m = {
            "xinv": np.ascontiguousarray(xinv[sl].T),
            "arep": np.ascontiguousarray(arep[:, sl]),
            "M": np.ascontiguousarray(M[c * nb_core:(c + 1) * nb_core]),
        }
        m.update(weights)
        in_maps.append(m)
        perms.append(perm[sl])
    return in_maps, perms, Es


# ---------------------------------------------------------------- builder

def _build(Es, reps=1):
    from contextlib import ExitStack
    from concourse import bacc, tile
    import concourse.mybir as mybir

    f32 = mybir.dt.float32
    f32r = mybir.dt.float32r
    bf16 = mybir.dt.bfloat16
    Silu = mybir.ActivationFunctionType.Silu
    Copy = mybir.ActivationFunctionType.Copy
    mult = mybir.AluOpType.mult
    add = mybir.AluOpType.add

    NT = Es // T
    nc = bacc.Bacc("TRN2", target_bir_lowering=False, debug=False,
                   num_devices=NCORES)

    din = lambda n, s, d=f32: nc.dram_tensor(n, s, d, kind="ExternalInput")
    xinv_d = din("xinv", [INV, Es])
    arep_d = din("arep", [BLK, Es])
    M_d = din("M", [Es // BLK, BLK, BLK], bf16)
    W1_d = din("W1", [INV, LAT])
    W2_d = din("W2", [LAT, LAT])
    We0_d = din("We0rep", [LAT, 2 * BLK])
    Wl1a_d = din("Wl1a", [LAT + F, LAT])
    Wl1b_d = din("Wl1b", [LAT, LAT])
    We1_d = din("We1rep", [LAT, BLK])
    Wf1_d = din("Wf1", [LAT + F, LAT])
    Wf2_d = din("Wf2", [LAT, LAT])
    L0_d = din("L0", [BLK, F])
    L1_d = din("L1", [BLK, F])
    Prep_d = din("Prep", [BLK, BLK])
    tpv_d = din("tpv", [BLK, 2])
    ident_d = din("ident", [BLK, BLK], bf16)
    out_d = nc.dram_tensor("out", [LAT, Es], f32, kind="ExternalOutput")

    with ExitStack() as ctx:
        tc = ctx.enter_context(tile.TileContext(nc))
        wp = ctx.enter_context(tc.tile_pool(name="w", bufs=1))
        sb = ctx.enter_context(tc.tile_pool(name="sb", bufs=2))
        ps = ctx.enter_context(tc.tile_pool(name="ps", bufs=5, space="PSUM"))
        pe = ctx.enter_context(tc.tile_pool(name="pe", bufs=3, space="PSUM"))

        def loadw_r(dram, K, N):
            """DMA f32 weight, then round to f32r chunks of <=128 rows."""
            tiles = []
            for k0 in range(0, K, 128):
                k1 = min(k0 + 128, K)
                raw = wp.tile([k1 - k0, N], f32, tag=f"raw{dram.name}{k0}",
                              name=f"raw{dram.name}{k0}")
                nc.sync.dma_start(raw[:], dram[k0:k1, :])
                t = wp.tile([k1 - k0, N], f32r, tag=f"w{dram.name}{k0}",
                            name=f"w{dram.name}{k0}")
                nc.vector.tensor_copy(t[:], raw[:])
                tiles.append(t)
            return tiles

        w1 = loadw_r(W1_d, INV, LAT)
        w2 = loadw_r(W2_d, LAT, LAT)
        we0 = loadw_r(We0_d, LAT, 2 * BLK)
        wl1a = loadw_r(Wl1a_d, LAT + F, LAT)
        wl1b = loadw_r(Wl1b_d, LAT, LAT)
        we1 = loadw_r(We1_d, LAT, BLK)
        wf1 = loadw_r(Wf1_d, LAT + F, LAT)
        wf2 = loadw_r(Wf2_d, LAT, LAT)
        L0 = loadw_r(L0_d, BLK, F)[0]
        L1 = loadw_r(L1_d, BLK, F)[0]
        Prep = loadw_r(Prep_d, BLK, BLK)[0]
        tpv = wp.tile([BLK, 2], f32, tag="tpv", name="tpv")
        nc.sync.dma_start(tpv[:], tpv_d[:])
        ident = wp.tile([BLK, BLK], bf16, tag="ident", name="ident")
        nc.sync.dma_start(ident[:], ident_d[:])

        def mm(wtiles, rhs_list, ncols, tag):
            """PSUM chunks of lhsT.T @ rhs accumulated over K chunks."""
            outs = []
            for j in range(0, ncols, 128):
                w = min(128, ncols - j)
                p = ps.tile([w, T], f32, tag="ps", name=f"{tag}{j}")
                for i, (wt, rt) in enumerate(zip(wtiles, rhs_list)):
                    nc.tensor.matmul(p[:], wt[:, j:j + w], rt,
                                     start=(i == 0), stop=(i == len(wtiles) - 1))
                outs.append(p)
            return outs

        def env_pass(ee, Mt, tag):
            """ee [128,T] bf16 (feature-major). Returns env [128,T] f32r
            = (B-I) @ ee along edges."""
            env = sb.tile([BLK, T], f32r, tag=f"{tag}env", name=f"{tag}env")
            for j in range(T // BLK):
                s = slice(j * BLK, (j + 1) * BLK)
                t1 = pe.tile([BLK, BLK], bf16, tag="pe", name=f"{tag}t1")
                nc.tensor.transpose(t1[:], ee[:, s], ident[:])
                em = sb.tile([BLK, BLK], bf16, tag=f"{tag}em", name=f"{tag}em")
                nc.scalar.activation(em[:], t1[:], Copy)
                mmp = pe.tile([BLK, BLK], f32, tag="pe", name=f"{tag}mm")
                nc.tensor.matmul(mmp[:], Mt[:, s], em[:], start=True, stop=True)
                bm = sb.tile([BLK, BLK], bf16, tag=f"{tag}bm", name=f"{tag}bm")
                nc.scalar.activation(bm[:], mmp[:], Copy)
                t2 = pe.tile([BLK, BLK], bf16, tag="pe", name=f"{tag}t2")
                nc.tensor.transpose(t2[:], bm[:], ident[:])
                nc.vector.tensor_copy(env[:, s], t2[:])
            return env

        for rep in range(reps):
          for it in range(NT):
            s = slice(it * T, (it + 1) * T)
            xtf = sb.tile([INV, T], f32, tag="xtf", name="xtf")
            nc.sync.dma_start(xtf[:], xinv_d[:, s])
            xt = sb.tile([INV, T], f32r, tag="xt", name="xt")
            nc.gpsimd.tensor_copy(xt[:], xtf[:])
            ar = sb.tile([BLK, T], f32, tag="ar", name="ar")
            nc.sync.dma_start(ar[:], arep_d[:, s])
            Mt = sb.tile([BLK, 4 * BLK], bf16, tag="Mt", name="Mt")
            for j in range(4):
                nc.sync.dma_start(Mt[:, j * BLK:(j + 1) * BLK], M_d[it * 4 + j])

            # ---- two-body latent ----
            h1p = mm(w1, [xt[:]], LAT, "h1")
            h1 = [sb.tile([128, T], f32r, tag=f"h1s{i}", name=f"h1s{i}")
                  for i in range(2)]
            for i in range(2):
                nc.scalar.activation(h1[i][:], h1p[i][:], Silu)
            l0p = mm(w2, [h1[0][:], h1[1][:]], LAT, "l0")
            lat0 = [sb.tile([128, T], f32r, tag=f"lat0{i}", name=f"lat0{i}")
                    for i in range(2)]
            for i in range(2):
                nc.scalar.activation(lat0[i][:], l0p[i][:], Copy)

            # ---- weights -> feat, env_e (layer 0) ----
            w0p = mm(we0, [lat0[0][:], lat0[1][:]], 2 * BLK, "w0")
            feat = sb.tile([BLK, T], f32r, tag="feat", name="feat")
            nc.vector.tensor_mul(feat[:], w0p[0][:], ar[:])
            ee0 = sb.tile([BLK, T], bf16, tag="ee0", name="ee0")
            nc.vector.tensor_mul(ee0[:], w0p[1][:], ar[:])
            env0 = env_pass(ee0, Mt, "e0")

            # ---- tp0 -> f1 [128,T] ----
            full = sb.tile([BLK, T], f32r, tag="full", name="full")
            nc.vector.tensor_mul(full[:], feat[:], env0[:])
            f1sp = mm([L0], [full[:]], F, "f1s")[0]
            s1rep = mm([Prep], [feat[:]], BLK, "s1rep")[0]
            s2rep = mm([Prep], [env0[:]], BLK, "s2rep")[0]
            f1 = sb.tile([BLK, T], f32r, tag="f1", name="f1")
            A = sb.tile([BLK, T], f32, tag="A", name="A")
            nc.vector.tensor_mul(A[:], feat[:], s2rep[:])
            B = sb.tile([BLK, T], f32, tag="B", name="B")
            nc.vector.tensor_mul(B[:], env0[:], s1rep[:])
            nc.vector.tensor_scalar_mul(f1[:], A[:], tpv[:, 0:1])
            nc.vector.scalar_tensor_tensor(f1[:], B[:], tpv[:, 1:2], f1[:],
                                           mult, add)
            nc.scalar.activation(f1[0:F], f1sp[:], Copy)

            # ---- layer-1 latent + resnet ----
            l1p = mm(wl1a, [lat0[0][:], lat0[1][:], f1[0:F]], LAT, "l1")
            h2 = [sb.tile([128, T], f32r, tag=f"h2s{i}", name=f"h2s{i}")
                  for i in range(2)]
            for i in range(2):
                nc.scalar.activation(h2[i][:], l1p[i][:], Silu)
            n1p = mm(wl1b, [h2[0][:], h2[1][:]], LAT, "n1")
            u1 = [sb.tile([128, T], f32r, tag=f"u1{i}", name=f"u1{i}")
                  for i in range(2)]
            for i in range(2):
                nc.vector.tensor_add(u1[i][:], lat0[i][:], n1p[i][:])

            # ---- env_w1 + env (layer 1) ----
            w1p = mm(we1, [u1[0][:], u1[1][:]], BLK, "wv1")[0]
            ee1 = sb.tile([BLK, T], bf16, tag="ee1", name="ee1")
            nc.vector.tensor_mul(ee1[:], w1p[:], ar[:])
            env1 = env_pass(ee1, Mt, "e1")

            # ---- tp1 -> gg [32,T] ----
            full1 = sb.tile([BLK, T], f32r, tag="full1", name="full1")
            nc.vector.tensor_mul(full1[:], f1[:], env1[:])
            ggp = mm([L1], [full1[:]], F, "gg")[0]
            gg = sb.tile([F, T], f32r, tag="gg", name="gg")
            nc.scalar.activation(gg[:], ggp[:], Copy)

            # ---- final latent ----
            fp = mm(wf1, [u1[0][:], u1[1][:], gg[:]], LAT, "fp")
            h3 = [sb.tile([128, T], f32r, tag=f"h3s{i}", name=f"h3s{i}")
                  for i in range(2)]
            for i in range(2):
                nc.scalar.activation(h3[i][:], fp[i][:], Silu)
            n3p = mm(wf2, [h3[0][:], h3[1][:]], LAT, "n3")
            for i in range(2):
                ob = sb.tile([128, T], f32, tag=f"ob{i}", name=f"ob{i}")
                nc.vector.scalar_tensor_tensor(ob[:], u1[i][:], 0.5, n3p[i][:],
                                               mult, add)
                nc.sync.dma_start(out_d[i * 128:(i + 1) * 128, s], ob[:])

    nc.compile()
    return nc


_CACHE = {}


def kernel(**inputs) -> np.ndarray:
    from concourse.bass_utils import run_bass_kernel_spmd

    in_maps, perms, Es = _prep_host(inputs)
    if Es not in _CACHE:
        _CACHE[Es] = _build(Es)
    nc = _CACHE[Es]

    res = run_bass_kernel_spmd(nc, in_maps, core_ids=list(range(NCORES)))
    E = np.asarray(inputs["edge_attr"]).shape[0]
    out = np.empty((E, LAT), dtype=np.float32)
    for c in range(NCORES):
        o = np.asarray(res.results[c]["out"])          # [LAT, Es]
        p = perms[c]
        v = p >= 0
        out[p[v]] = o.T[v]
    return out


# revision 11
# speedup vs baseline: 1.4898x; 1.4898x over previous
"""Allegro-style GNN edge kernel on 8 TRN2 NeuronCores.

Strategy: the reference only ever uses edge_index[0] (edge_center) — the env
scatter-sum groups edges by center atom. We sort edges by center on the host
and pack whole atom-runs into 128-edge blocks (an atom's edges never straddle
a block). Blocks are dealt to the 8 cores, so the segment-sum + gather
becomes, per 128-edge block, multiplication by a block-diagonal 0/1 matrix
(B - I), built on the host — fully core-local, zero collectives.

Device layout is feature-major ([feat_dim partitions, edges free]); all MLP
matmuls use float32r (full-rate fp32, tf32-class precision) with weights
stationary. Cross-partition-block reductions/broadcasts in the tensor
products run as matmuls against small host-built constant matrices (engines
require partition-aligned operands). The (B-I) env matmul runs in bf16.
"""

import sys
import numpy as np

sys.path.insert(0, "/opt/trn_rl_repo")

N_ATOMS = 16000
F = 32
INV = 64
LAT = 256
NCORES = 8
T = 512          # edges per compute tile (matmul moving dim)
BLK = 128        # edges per env block
SQRT3 = 3.0 ** 0.5
RSQRT2 = 2.0 ** -0.5
ENV_C = float(1.0 / np.sqrt(16.0 - 1.0))


# ---------------------------------------------------------------- host prep

def _pack_edges(edge_center):
    """Sort edges by center atom; pack whole atom-runs into 128-edge blocks."""
    E = edge_center.shape[0]
    counts = np.bincount(edge_center, minlength=N_ATOMS)
    order_sorted = np.argsort(edge_center, kind="stable")

    atom_block = np.zeros(N_ATOMS, dtype=np.int64)
    atom_off = np.zeros(N_ATOMS, dtype=np.int64)
    order_by_cnt = np.argsort(-counts, kind="stable")
    fills = []                     # best-fit decreasing via capacity buckets
    buckets = [[] for _ in range(BLK + 1)]   # buckets[r] = blocks with r free
    for a in order_by_cnt:
        c = int(counts[a])
        if c == 0:
            continue
        if c > BLK:
            raise ValueError(f"atom {a} has {c} > {BLK} edges")
        for r in range(c, BLK + 1):
            if buckets[r]:
                bi = buckets[r].pop()
                break
        else:
            bi = len(fills)
            fills.append(0)
            r = BLK
        atom_block[a] = bi
        atom_off[a] = fills[bi]
        fills[bi] += c
        buckets[r - c].append(bi)
    n_blocks = len(fills)

    cum = np.concatenate([[0], np.cumsum(counts)])[:-1]
    run_start = np.repeat(cum, counts)
    within = np.arange(E) - run_start
    sc = edge_center[order_sorted]
    slot = atom_block[sc] * BLK + atom_off[sc] + within
    return slot, order_sorted, n_blocks


def _prep_host(inputs):
    """Shard + reorder inputs. Returns (in_maps, perms, Es)."""
    import ml_dtypes

    edge_center = np.asarray(inputs["edge_index"])[0].astype(np.int64)
    edge_attr = np.asarray(inputs["edge_attr"], dtype=np.float32)
    edge_inv = np.asarray(inputs["edge_inv"], dtype=np.float32)

    slot, order_sorted, n_blocks = _pack_edges(edge_center)

    nb_core = -(-n_blocks // NCORES)            # ceil
    nb_core = -(-nb_core // 16) * 16            # multiple of 16 (outer tile)
    Es = nb_core * BLK
    nb_pad = nb_core * NCORES
    total = nb_pad * BLK

    perm = np.full(total, -1, dtype=np.int64)   # slot -> original edge id
    perm[slot] = order_sorted
    centers = np.full(total, -1, dtype=np.int64)
    centers[slot] = edge_center[order_sorted]

    xinv = np.zeros((total, INV), dtype=np.float32)
    attr = np.zeros((total, 4), dtype=np.float32)
    valid = perm >= 0
    xinv[valid] = edge_inv[perm[valid]]
    attr[valid] = edge_attr[perm[valid]]

    # attr replicated across the 32 features: row 32c+f = attr[:, c]
    arep = np.repeat(attr.T, F, axis=0)          # [128, total]

    # per-block (B - I) matrices, bf16
    cb = centers.reshape(nb_pad, BLK)
    M = (cb[:, :, None] == cb[:, None, :])
    M &= (cb >= 0)[:, :, None]
    M = M.astype(np.float32)
    idx = np.arange(BLK)
    M[:, idx, idx] = 0.0
    M = M.astype(ml_dtypes.bfloat16)

    # ---- weights: fold fan-in norms, ENV_C, 1/sqrt3, resnet RSQRT2 ----
    g = lambda k: np.asarray(inputs[k], dtype=np.float32)
    r = RSQRT2
    W1 = g("W_tb1") / np.sqrt(INV)
    W2 = g("W_tb2") / np.sqrt(LAT)
    We0 = g("W_env0") / np.sqrt(LAT)
    ew0, ew1 = We0[:, 0:64:2], We0[:, 1:64:2]
    nw0, nw1 = We0[:, 64::2], We0[:, 65::2]
    We0rep = np.concatenate([ew0, ew1, ew1, ew1, nw0, nw1, nw1, nw1], axis=1)
    Wl1a = g("W_l1a") / np.sqrt(LAT + F)
    Wl1b = g("W_l1b") / np.sqrt(LAT)
    We1 = g("W_env1") / np.sqrt(LAT) * r
    m0, m1 = We1[:, 0::2], We1[:, 1::2]
    We1rep = np.concatenate([m0, m1, m1, m1], axis=1)
    Wf1 = g("W_f1") / np.sqrt(LAT + F)
    Wf1 = Wf1.copy()
    Wf1[:LAT] *= r                               # lat input arrives unscaled (u1)
    Wf2 = g("W_f2") / np.sqrt(LAT) * r

    NBO = 16                                     # blocks per outer tile
    wtp0 = g("w_tp0")                            # [4, F]
    wtp1 = g("w_tp1")                            # [2, F]
    idf = np.arange(F)
    L0 = np.zeros((BLK, F), dtype=np.float32)
    L0[idf, idf] = wtp0[0] * ENV_C
    for c in range(1, 4):
        L0[c * F + idf, idf] = wtp0[1] * ENV_C / SQRT3
    L1 = np.zeros((BLK, F), dtype=np.float32)
    L1[idf, idf] = wtp1[0] * ENV_C
    for c in range(1, 4):
        L1[c * F + idf, idf] = wtp1[1] * ENV_C / SQRT3
    Prep = np.zeros((BLK, BLK), dtype=np.float32)
    for c in range(4):
        Prep[idf, c * F + idf] = 1.0
    tpv = np.zeros((BLK, 2), dtype=np.float32)   # col0=w3', col1=w2'
    for c in range(1, 4):
        tpv[c * F + idf, 0] = wtp0[3] * ENV_C
        tpv[c * F + idf, 1] = wtp0[2] * ENV_C
    bf = ml_dtypes.bfloat16
    weights = {
        "W1": W1, "W2": W2, "We0rep": We0rep,
        "Wl1a": Wl1a[:LAT], "Wl1aS": Wl1a[LAT:].astype(bf),
        "Wl1b": Wl1b, "We1rep": We1rep,
        "Wf1": Wf1[:LAT], "Wf1S": Wf1[LAT:].astype(bf), "Wf2": Wf2,
        "L0": L0.astype(bf), "L1": L1.astype(bf), "Prep": Prep.astype(bf),
        "tpv": tpv,
    }
    weights = {k: np.ascontiguousarray(v) for k, v in weights.items()}

    in_maps, perms = [], []
    for c in range(NCORES):
        sl = slice(c * Es, (c + 1) * Es)
        Mc = M[c * nb_core:(c + 1) * nb_core]
        M2 = Mc.reshape(nb_core // NBO, NBO, BLK, BLK).transpose(0, 2, 1, 3)
        M2 = M2.reshape(nb_core // NBO, BLK, NBO * BLK)
        m = {
            "xinv": np.ascontiguousarray(xinv[sl].T),
            "arep": np.ascontiguousarray(arep[:, sl]),
            "M": np.ascontiguousarray(M2),
        }
        m.update(weights)
        in_maps.append(m)
        perms.append(perm[sl])
    return in_maps, perms, Es


# ---------------------------------------------------------------- builder

def _build(Es, reps=1):
    from contextlib import ExitStack
    from concourse import bacc, tile
    import concourse.mybir as mybir

    f32 = mybir.dt.float32
    f32r = mybir.dt.float32r
    bf16 = mybir.dt.bfloat16
    Silu = mybir.ActivationFunctionType.Silu
    Copy = mybir.ActivationFunctionType.Copy
    mult = mybir.AluOpType.mult
    add = mybir.AluOpType.add

    OUT = 4 * T                                  # 2048-edge outer tile
    NBO = OUT // BLK                             # 16 env blocks per outer
    NTO = Es // OUT
    nc = bacc.Bacc("TRN2", target_bir_lowering=False, debug=False,
                   num_devices=NCORES)

    din = lambda n, s, d=f32: nc.dram_tensor(n, s, d, kind="ExternalInput")
    xinv_d = din("xinv", [INV, Es])
    arep_d = din("arep", [BLK, Es])
    M_d = din("M", [NTO, BLK, NBO * BLK], bf16)
    W1_d = din("W1", [INV, LAT])
    W2_d = din("W2", [LAT, LAT])
    We0_d = din("We0rep", [LAT, 2 * BLK])
    Wl1a_d = din("Wl1a", [LAT, LAT])
    Wl1aS_d = din("Wl1aS", [F, LAT], bf16)
    Wl1b_d = din("Wl1b", [LAT, LAT])
    We1_d = din("We1rep", [LAT, BLK])
    Wf1_d = din("Wf1", [LAT, LAT])
    Wf1S_d = din("Wf1S", [F, LAT], bf16)
    Wf2_d = din("Wf2", [LAT, LAT])
    L0_d = din("L0", [BLK, F], bf16)
    L1_d = din("L1", [BLK, F], bf16)
    Prep_d = din("Prep", [BLK, BLK], bf16)
    tpv_d = din("tpv", [BLK, 2])
    out_d = nc.dram_tensor("out", [LAT, Es], f32, kind="ExternalOutput")

    with ExitStack() as ctx:
        tc = ctx.enter_context(tile.TileContext(nc))
        wp = ctx.enter_context(tc.tile_pool(name="w", bufs=1))
        so = ctx.enter_context(tc.tile_pool(name="so", bufs=1))   # outer tiles
        si = ctx.enter_context(tc.tile_pool(name="si", bufs=2))   # inner/stream
        ps = ctx.enter_context(tc.tile_pool(name="ps", bufs=5, space="PSUM"))
        pv = ctx.enter_context(tc.tile_pool(name="pv", bufs=2, space="PSUM"))

        def loadw_r(dram, K, N):
            tiles = []
            for k0 in range(0, K, 128):
                k1 = min(k0 + 128, K)
                raw = wp.tile([k1 - k0, N], f32, tag=f"raw{dram.name}{k0}",
                              name=f"raw{dram.name}{k0}")
                nc.sync.dma_start(raw[:], dram[k0:k1, :])
                t = wp.tile([k1 - k0, N], f32r, tag=f"w{dram.name}{k0}",
                            name=f"w{dram.name}{k0}")
                nc.vector.tensor_copy(t[:], raw[:])
                tiles.append(t)
            return tiles

        def loadw_bf(dram, K, N):
            t = wp.tile([K, N], bf16, tag=f"w{dram.name}", name=f"w{dram.name}")
            nc.sync.dma_start(t[:], dram[:])
            return t

        w1 = loadw_r(W1_d, INV, LAT)
        w2 = loadw_r(W2_d, LAT, LAT)
        we0 = loadw_r(We0_d, LAT, 2 * BLK)
        wl1a = loadw_r(Wl1a_d, LAT, LAT) + [loadw_bf(Wl1aS_d, F, LAT)]
        wl1b = loadw_r(Wl1b_d, LAT, LAT)
        we1 = loadw_r(We1_d, LAT, BLK)
        wf1 = loadw_r(Wf1_d, LAT, LAT) + [loadw_bf(Wf1S_d, F, LAT)]
        wf2 = loadw_r(Wf2_d, LAT, LAT)
        L0 = loadw_bf(L0_d, BLK, F)
        L1 = loadw_bf(L1_d, BLK, F)
        Prep = loadw_bf(Prep_d, BLK, BLK)
        tpv = wp.tile([BLK, 2], f32, tag="tpv", name="tpv")
        nc.sync.dma_start(tpv[:], tpv_d[:])

        def mm(wtiles, rhs_list, ncols, tag):
            outs = []
            for j in range(0, ncols, 128):
                w = min(128, ncols - j)
                p = ps.tile([w, T], f32, tag="ps", name=f"{tag}{j}")
                for i, (wt, rt) in enumerate(zip(wtiles, rhs_list)):
                    nc.tensor.matmul(p[:], wt[:, j:j + w], rt,
                                     start=(i == 0), stop=(i == len(wtiles) - 1))
                outs.append(p)
            return outs

        def env_pass(ee, Mt, env, it, tag):
            """ee [128,T] bf16 inner tile -> env[:, it*T:(it+1)*T] = (B-I)@ee."""
            emp = pv.tile([BLK, T], f32, tag="pv", name=f"{tag}p")
            for j in range(4):
                sj = slice(j * BLK, (j + 1) * BLK)
                em = si.tile([BLK, BLK], bf16, tag=f"{tag}em", name=f"{tag}em")
                nc.sync.dma_start(em[:], ee[:, sj], transpose=True)
                gb = it * 4 + j
                nc.tensor.matmul(emp[:, sj], Mt[:, gb * BLK:(gb + 1) * BLK],
                                 em[:], start=True, stop=True)
            bm = si.tile([BLK, T], bf16, tag=f"{tag}bm", name=f"{tag}bm")
            nc.scalar.activation(bm[:], emp[:], Copy)
            for j in range(4):
                sj = slice(j * BLK, (j + 1) * BLK)
                nc.sync.dma_start(env[:, it * T + j * BLK:it * T + (j + 1) * BLK],
                                  bm[:, sj], transpose=True)

        for rep in range(reps):
          for ot in range(NTO):
            so_ = slice(ot * OUT, (ot + 1) * OUT)
            xtf = si.tile([INV, OUT], f32, tag="xtf", name="xtf")
            nc.sync.dma_start(xtf[:], xinv_d[:, so_])
            xt = si.tile([INV, OUT], f32r, tag="xt", name="xt")
            nc.gpsimd.tensor_copy(xt[:], xtf[:])
            ar = si.tile([BLK, OUT], f32, tag="ar", name="ar")
            nc.sync.dma_start(ar[:], arep_d[:, so_])
            Mt = si.tile([BLK, NBO * BLK], bf16, tag="Mt", name="Mt")
            nc.sync.dma_start(Mt[:], M_d[ot])

            lat0 = [so.tile([128, OUT], f32r, tag=f"lat0{i}", name=f"lat0{i}")
                    for i in range(2)]
            u1 = [so.tile([128, OUT], f32r, tag=f"u1{i}", name=f"u1{i}")
                  for i in range(2)]
            feat = so.tile([BLK, OUT], bf16, tag="feat", name="feat")
            env0 = so.tile([BLK, OUT], bf16, tag="env0", name="env0")
            env1 = so.tile([BLK, OUT], bf16, tag="env1", name="env1")
            f1 = so.tile([BLK, OUT], bf16, tag="f1", name="f1")
            A = so.tile([BLK, OUT], bf16, tag="A", name="A")
            B = so.tile([BLK, OUT], bf16, tag="B", name="B")
            full = so.tile([BLK, OUT], bf16, tag="full", name="full")
            full1 = so.tile([BLK, OUT], bf16, tag="full1", name="full1")
            scal = so.tile([F, OUT], bf16, tag="scal", name="scal")
            gg = so.tile([F, OUT], bf16, tag="gg", name="gg")
            ob = [so.tile([128, OUT], f32, tag=f"ob{i}", name=f"ob{i}")
                  for i in range(2)]

            ssl = [slice(it * T, (it + 1) * T) for it in range(4)]

            # phase A: two-body latent, weights, env_e0, env0
            for it in range(4):
                sl = ssl[it]
                h1p = mm(w1, [xt[:, sl]], LAT, "h1")
                h1 = [si.tile([128, T], f32r, tag=f"h1s{i}", name=f"h1s{i}")
                      for i in range(2)]
                for i in range(2):
                    nc.scalar.activation(h1[i][:], h1p[i][:], Silu)
                l0p = mm(w2, [h1[0][:], h1[1][:]], LAT, "l0")
                for i in range(2):
                    nc.scalar.activation(lat0[i][:, sl], l0p[i][:], Copy)
                w0p = mm(we0, [lat0[0][:, sl], lat0[1][:, sl]], 2 * BLK, "w0")
                nc.vector.tensor_mul(feat[:, sl], w0p[0][:], ar[:, sl])
                ee0 = si.tile([BLK, T], bf16, tag="ee0", name="ee0")
                nc.vector.tensor_mul(ee0[:], w0p[1][:], ar[:, sl])
                env_pass(ee0, Mt, env0, it, "e0")

            # phase B/C: tp0
            nc.vector.tensor_mul(full[:], feat[:], env0[:])
            for it in range(4):
                sl = ssl[it]
                f1sp = mm([L0], [full[:, sl]], F, "f1s")[0]
                nc.scalar.activation(scal[:, sl], f1sp[:], Copy)
                s1p = mm([Prep], [feat[:, sl]], BLK, "s1rep")[0]
                s2p = mm([Prep], [env0[:, sl]], BLK, "s2rep")[0]
                nc.vector.tensor_mul(A[:, sl], feat[:, sl], s2p[:])
                nc.vector.tensor_mul(B[:, sl], env0[:, sl], s1p[:])
            nc.vector.tensor_scalar_mul(f1[:], A[:], tpv[:, 0:1])
            nc.vector.scalar_tensor_tensor(f1[:], B[:], tpv[:, 1:2], f1[:],
                                           mult, add)
            nc.vector.tensor_copy(f1[0:F], scal[:])

            # phase E/F: layer-1 latent, resnet, env1
            for it in range(4):
                sl = ssl[it]
                l1p = mm(wl1a, [lat0[0][:, sl], lat0[1][:, sl], scal[:, sl]],
                         LAT, "l1")
                h2 = [si.tile([128, T], f32r, tag=f"h2s{i}", name=f"h2s{i}")
                      for i in range(2)]
                for i in range(2):
                    nc.scalar.activation(h2[i][:], l1p[i][:], Silu)
                n1p = mm(wl1b, [h2[0][:], h2[1][:]], LAT, "n1")
                for i in range(2):
                    nc.vector.tensor_add(u1[i][:, sl], lat0[i][:, sl], n1p[i][:])
                w1p = mm(we1, [u1[0][:, sl], u1[1][:, sl]], BLK, "wv1")[0]
                ee1 = si.tile([BLK, T], bf16, tag="ee1", name="ee1")
                nc.vector.tensor_mul(ee1[:], w1p[:], ar[:, sl])
                env_pass(ee1, Mt, env1, it, "e1")

            # phase G/H: tp1
            nc.vector.tensor_mul(full1[:], f1[:], env1[:])
            for it in range(4):
                sl = ssl[it]
                ggp = mm([L1], [full1[:, sl]], F, "gg")[0]
                nc.scalar.activation(gg[:, sl], ggp[:], Copy)

            # phase I: final latent
            for it in range(4):
                sl = ssl[it]
                fp = mm(wf1, [u1[0][:, sl], u1[1][:, sl], gg[:, sl]], LAT, "fp")
                h3 = [si.tile([128, T], f32r, tag=f"h3s{i}", name=f"h3s{i}")
                      for i in range(2)]
                for i in range(2):
                    nc.scalar.activation(h3[i][:], fp[i][:], Silu)
                n3p = mm(wf2, [h3[0][:], h3[1][:]], LAT, "n3")
                for i in range(2):
                    nc.vector.scalar_tensor_tensor(ob[i][:, sl], u1[i][:, sl],
                                                   0.5, n3p[i][:], mult, add)
            for i in range(2):
                nc.sync.dma_start(out_d[i * 128:(i + 1) * 128, so_], ob[i][:])

    nc.compile()
    return nc


_CACHE = {}


def kernel(**inputs) -> np.ndarray:
    from concourse.bass_utils import run_bass_kernel_spmd

    in_maps, perms, Es = _prep_host(inputs)
    if Es not in _CACHE:
        _CACHE[Es] = _build(Es)
    nc = _CACHE[Es]

    res = run_bass_kernel_spmd(nc, in_maps, core_ids=list(range(NCORES)))
    E = np.asarray(inputs["edge_attr"]).shape[0]
    out = np.empty((E, LAT), dtype=np.float32)
    for c in range(NCORES):
        o = np.asarray(res.results[c]["out"])          # [LAT, Es]
        p = perms[c]
        v = p >= 0
        out[p[v]] = o.T[v]
    return out
